# revision 4
# baseline (speedup 1.0000x reference)
"""CISSVAE (cluster-routed VAE) Trainium2 kernel.

Strategy: expert-parallel over the 8 clusters — core c handles exactly the rows
with cluster_labels == c (capacity-padded to a fixed CAP so all 8 cores run one
SPMD program). Host does the routing (gather by cluster, pad, transpose to
feature-major) and the inverse scatter. On-device everything is dense matmuls:

    h0 = relu(W_enc0[c]^T x + b)   [1024, CAP]
    h1 = relu(W_enc1^T h0 + b)     [512, CAP]
    mu = W_mu^T h1 + b             [128, CAP]
    lv = W_logvar^T h1 + b         [128, CAP]
    z  = mu + eps * exp(0.5 lv)    [128, CAP]
    d0 = relu(W_dec0^T z + b)      [512, CAP]
    d1 = relu(W_dec1[c]^T d0 + b)  [1024, CAP]
    recon = W_out^T d1 + b         [2048, CAP]

Activations live feature-major ([feature, column]) so every layer is
out[o, n] += W[k, o] * act[k, n] with W slices as the stationary operand --
no transposes anywhere on device. Matmuls run in float32r (TF32-like, full
PE rate for moving dim >= 256), accumulation in fp32 PSUM, bias+activation
fused into the ScalarE PSUM evacuation.
"""

import math
from contextlib import ExitStack

import numpy as np

import concourse.bacc as bacc
import concourse.mybir as mybir
import concourse.tile as tile
from concourse.bass_utils import run_bass_kernel_spmd

F32 = mybir.dt.float32
F32R = mybir.dt.float32r
AF = mybir.ActivationFunctionType

D_IN, H0, H1, LAT, C = 2048, 1024, 512, 128, 8
N_CORES = 8
P = 128

_program_cache: dict = {}


def _nchunks(cap):
    """Split cap columns into chunks of 384/512 (>=256 keeps f32r at full rate)."""
    if cap % 512 == 0:
        size = 512
    elif cap % 384 == 0:
        size = 384
    elif cap % 256 == 0:
        size = 256
    else:
        raise ValueError(f"cap {cap} not a multiple of 256/384/512")
    return [(i * size, size) for i in range(cap // size)]


def _build(cap):
    nbs = _nchunks(cap)
    nc = bacc.Bacc(trn_type="TRN2", target_bir_lowering=False, debug=False)

    xT = nc.dram_tensor("xT", [D_IN, cap], F32R, kind="ExternalInput")
    epsT = nc.dram_tensor("epsT", [LAT, cap], F32, kind="ExternalInput")
    w_enc0 = nc.dram_tensor("w_enc0", [D_IN, H0], F32R, kind="ExternalInput")
    b_enc0 = nc.dram_tensor("b_enc0", [H0], F32, kind="ExternalInput")
    w_enc1 = nc.dram_tensor("w_enc1", [H0, H1], F32R, kind="ExternalInput")
    b_enc1 = nc.dram_tensor("b_enc1", [H1], F32, kind="ExternalInput")
    w_mu = nc.dram_tensor("w_mu", [H1, LAT], F32R, kind="ExternalInput")
    b_mu = nc.dram_tensor("b_mu", [LAT], F32, kind="ExternalInput")
    w_lv = nc.dram_tensor("w_lv", [H1, LAT], F32R, kind="ExternalInput")
    b_lv = nc.dram_tensor("b_lv", [LAT], F32, kind="ExternalInput")
    w_dec0 = nc.dram_tensor("w_dec0", [LAT, H1], F32R, kind="ExternalInput")
    b_dec0 = nc.dram_tensor("b_dec0", [H1], F32, kind="ExternalInput")
    w_dec1 = nc.dram_tensor("w_dec1", [H1, H0], F32R, kind="ExternalInput")
    b_dec1 = nc.dram_tensor("b_dec1", [H0], F32, kind="ExternalInput")
    w_out = nc.dram_tensor("w_out", [H0, D_IN], F32R, kind="ExternalInput")
    b_out = nc.dram_tensor("b_out", [D_IN], F32, kind="ExternalInput")

    reconT = nc.dram_tensor("reconT", [D_IN, cap], F32, kind="ExternalOutput")
    muT = nc.dram_tensor("muT", [LAT, cap], F32, kind="ExternalOutput")
    lvT = nc.dram_tensor("lvT", [LAT, cap], F32, kind="ExternalOutput")

    with tile.TileContext(nc) as tc, ExitStack() as ctx:
        data = ctx.enter_context(tc.tile_pool(name="data", bufs=1))
        wstream = ctx.enter_context(tc.tile_pool(name="wstream", bufs=2))
        stage = ctx.enter_context(tc.tile_pool(name="stage", bufs=4))
        psum = ctx.enter_context(tc.tile_pool(name="psum", bufs=8, space="PSUM"))

        def load_bias(b_dram, O):
            ot = O // P
            bt = data.tile([P, ot], F32, name=f"bias_{b_dram.name}")
            nc.sync.dma_start(bt[:], b_dram.rearrange("(t p) -> p t", p=P))
            return bt

        def load_weight(w_dram, K, O, tag=None):
            # SBUF [P, KT, O]; per partition KT chunks of O contiguous floats.
            kt = K // P
            wt = data.tile([P, kt, O], F32R, name=f"w_{w_dram.name}", tag=tag or f"w_{w_dram.name}")
            nc.sync.dma_start(wt[:], w_dram.rearrange("(kt kp) o -> kp kt o", kp=P))
            return wt

        bt_enc0 = load_bias(b_enc0, H0)
        bt_enc1 = load_bias(b_enc1, H1)
        bt_mu = load_bias(b_mu, LAT)
        bt_lv = load_bias(b_lv, LAT)
        bt_dec0 = load_bias(b_dec0, H1)
        bt_dec1 = load_bias(b_dec1, H0)
        bt_out = load_bias(b_out, D_IN)

        # x resident, loaded per k-tile
        KT_X = D_IN // P
        xt = data.tile([P, KT_X, cap], F32R, tag="slabA")
        xT_r = xT.rearrange("(kt kp) n -> kp kt n", kp=P)
        for k in range(KT_X):
            nc.sync.dma_start(xt[:, k, :], xT_r[:, k, :])

        def dense_streamed(inp, K, O, w_dram, bt, act, out_dt, name, tag=None):
            """Weights streamed per 128-wide o-slice; returns [P, OT, cap] tile."""
            KT, OT = K // P, O // P
            out = data.tile([P, OT, cap], out_dt, name=name, tag=tag or name)
            w_r = w_dram.rearrange("(kt kp) o -> kp kt o", kp=P)
            for o in range(OT):
                wt = wstream.tile([P, KT, P], F32R, tag="wstream", name=f"{name}_w{o}")
                nc.sync.dma_start(wt[:], w_r[:, :, o * P : (o + 1) * P])
                for n0, nb in nbs:
                    ps = psum.tile([P, 512], F32, tag="ps", name=f"{name}_ps{o}")
                    for k in range(KT):
                        nc.tensor.matmul(
                            ps[:, :nb],
                            wt[:, k, :],
                            inp[:, k, n0 : n0 + nb],
                            start=(k == 0),
                            stop=(k == KT - 1),
                        )
                    nc.scalar.activation(
                        out[:, o, n0 : n0 + nb], ps[:, :nb], act, bias=bt[:, o : o + 1]
                    )
            return out

        def dense_resident(inp, K, O, wt_sb, bt, act, out_dt, name, tag=None):
            """Weights already in SBUF as [P, KT, O]; returns [P, OT, cap]."""
            KT, OT = K // P, O // P
            out = data.tile([P, OT, cap], out_dt, name=name, tag=tag or name)
            for o in range(OT):
                for n0, nb in nbs:
                    ps = psum.tile([P, 512], F32, tag="ps", name=f"{name}_ps{o}")
                    for k in range(KT):
                        nc.tensor.matmul(
                            ps[:, :nb],
                            wt_sb[:, k, o * P : (o + 1) * P],
                            inp[:, k, n0 : n0 + nb],
                            start=(k == 0),
                            stop=(k == KT - 1),
                        )
                    nc.scalar.activation(
                        out[:, o, n0 : n0 + nb], ps[:, :nb], act, bias=bt[:, o : o + 1]
                    )
            return out

        # encoder
        h0 = dense_streamed(xt, D_IN, H0, w_enc0, bt_enc0, AF.Relu, F32R, "h0")
        wt_enc1 = load_weight(w_enc1, H0, H1, tag="wres")
        h1 = dense_resident(h0, H0, H1, wt_enc1, bt_enc1, AF.Relu, F32R, "h1")

        # latent heads (fp32 outputs; also DMA'd out)
        wt_mu = load_weight(w_mu, H1, LAT)
        wt_lv = load_weight(w_lv, H1, LAT)
        mu_sb = dense_resident(h1, H1, LAT, wt_mu, bt_mu, AF.Identity, F32, "mu_sb")
        lv_sb = dense_resident(h1, H1, LAT, wt_lv, bt_lv, AF.Identity, F32, "lv_sb")
        nc.sync.dma_start(muT[:, :], mu_sb[:, 0, :])
        nc.sync.dma_start(lvT[:, :], lv_sb[:, 0, :])

        # z = mu + eps * exp(0.5 * lv)
        eps_sb = data.tile([P, cap], F32)
        nc.sync.dma_start(eps_sb[:], epsT[:, :])
        t_sb = data.tile([P, cap], F32)
        nc.scalar.activation(t_sb[:], lv_sb[:, 0, :], AF.Exp, scale=0.5)
        z_sb = data.tile([P, 1, cap], F32R)
        nc.vector.scalar_tensor_tensor(
            z_sb[:, 0, :],
            eps_sb[:],
            1.0,
            t_sb[:],
            mybir.AluOpType.mult,
            mybir.AluOpType.mult,
        )
        nc.vector.tensor_add(z_sb[:, 0, :], z_sb[:, 0, :], mu_sb[:, 0, :])

        # decoder
        wt_dec0 = load_weight(w_dec0, LAT, H1)
        d0 = dense_resident(z_sb, LAT, H1, wt_dec0, bt_dec0, AF.Relu, F32R, "d0", tag="h1")
        wt_dec1 = load_weight(w_dec1, H1, H0, tag="wres")
        d1 = dense_resident(d0, H1, H0, wt_dec1, bt_dec1, AF.Relu, F32R, "d1", tag="slabA")

        # output layer: streamed weights, staged through SBUF then DMA out
        KT, OT = H0 // P, D_IN // P
        w_r = w_out.rearrange("(kt kp) o -> kp kt o", kp=P)
        for o in range(OT):
            wt = wstream.tile([P, KT, P], F32R, tag="wstream", name=f"wout{o}")
            nc.sync.dma_start(wt[:], w_r[:, :, o * P : (o + 1) * P])
            for n0, nb in nbs:
                ps = psum.tile([P, 512], F32, tag="ps", name=f"out_ps{o}")
                for k in range(KT):
                    nc.tensor.matmul(
                        ps[:, :nb],
                        wt[:, k, :],
                        d1[:, k, n0 : n0 + nb],
                        start=(k == 0),
                        stop=(k == KT - 1),
                    )
                st = stage.tile([P, 512], F32, tag="ostage", name=f"out_st{o}")
                nc.scalar.activation(
                    st[:, :nb], ps[:, :nb], AF.Identity, bias=bt_out[:, o : o + 1]
                )
                nc.sync.dma_start(
                    reconT[o * P : (o + 1) * P, n0 : n0 + nb], st[:, :nb]
                )

    nc.compile()
    return nc


def _get_program(cap):
    if cap not in _program_cache:
        _program_cache[cap] = _build(cap)
    return _program_cache[cap]


def _route(cluster_labels):
    labels = np.asarray(cluster_labels).astype(np.int64)
    idx = [np.nonzero(labels == c)[0] for c in range(C)]
    max_n = max((len(i) for i in idx), default=1)
    cap = max(256, int(math.ceil(max_n / 128.0)) * 128)
    while cap % 512 and cap % 384 and cap % 256:
        cap += 128
    idx_pad = []
    for c in range(C):
        i = idx[c]
        if len(i) == 0:
            pad = np.zeros(cap, dtype=np.int64)
        else:
            pad = np.concatenate([i, np.full(cap - len(i), i[0], dtype=np.int64)])
        idx_pad.append(pad)
    return idx, idx_pad, cap


def _run(inputs, trace=False):
    x = np.ascontiguousarray(np.asarray(inputs["x"], dtype=np.float32))
    eps = np.ascontiguousarray(np.asarray(inputs["eps"], dtype=np.float32))
    idx, idx_pad, cap = _route(inputs["cluster_labels"])
    nc = _get_program(cap)

    f32c = lambda a: np.ascontiguousarray(np.asarray(a, dtype=np.float32))
    shared = {
        "w_enc1": f32c(inputs["W_enc1"]),
        "b_enc1": f32c(inputs["b_enc1"]),
        "w_mu": f32c(inputs["W_mu"]),
        "b_mu": f32c(inputs["b_mu"]),
        "w_lv": f32c(inputs["W_logvar"]),
        "b_lv": f32c(inputs["b_logvar"]),
        "w_dec0": f32c(inputs["W_dec0"]),
        "b_dec0": f32c(inputs["b_dec0"]),
        "w_out": f32c(inputs["W_out"]),
        "b_out": f32c(inputs["b_out"]),
    }
    in_maps = []
    for c in range(C):
        ip = idx_pad[c]
        in_maps.append(
            {
                "xT": np.ascontiguousarray(x[ip].T),
                "epsT": np.ascontiguousarray(eps[ip].T),
                "w_enc0": f32c(inputs["W_enc0"][c]),
                "b_enc0": f32c(inputs["b_enc0"][c]),
                "w_dec1": f32c(inputs["W_dec1"][c]),
                "b_dec1": f32c(inputs["b_dec1"][c]),
                **shared,
            }
        )

    res = run_bass_kernel_spmd(nc, in_maps, core_ids=list(range(N_CORES)), trace=trace)

    B = x.shape[0]
    recon = np.empty((B, D_IN), dtype=np.float32)
    mu = np.empty((B, LAT), dtype=np.float32)
    logvar = np.empty((B, LAT), dtype=np.float32)
    for c in range(C):
        i = idx[c]
        if len(i) == 0:
            continue
        r = res.results[c]
        recon[i] = r["reconT"][:, : len(i)].T
        mu[i] = r["muT"][:, : len(i)].T
        logvar[i] = r["lvT"][:, : len(i)].T
    return (recon, mu, logvar), res


def kernel(**inputs):
    outs, _ = _run(inputs, trace=False)
    return outs


# revision 7
# speedup vs baseline: 1.1162x; 1.1162x over previous
"""CISSVAE (cluster-routed VAE) Trainium2 kernel.

Strategy: expert-parallel over the 8 clusters — core c handles exactly the rows
with cluster_labels == c (capacity-padded to a fixed CAP so all 8 cores run one
SPMD program). Host does the routing (gather by cluster, pad, transpose to
feature-major) and the inverse scatter. On-device everything is dense matmuls:

    h0 = relu(W_enc0[c]^T x + b)   [1024, CAP]
    h1 = relu(W_enc1^T h0 + b)     [512, CAP]
    lv = W_logvar^T h1 + b         [128, CAP]
    mu = W_mu^T h1 + b             [128, CAP]
    z  = mu + eps * exp(0.5 lv)    [128, CAP]
    d0 = relu(W_dec0^T z + b)      [512, CAP]
    d1 = relu(W_dec1[c]^T d0 + b)  [1024, CAP]
    recon = W_out^T d1 + b         [2048, CAP]

Activations live feature-major ([feature, column]) so every layer is
out[o, n] += W[k, o] * act[k, n] with W slices as the stationary operand —
no transposes anywhere on device. Matmuls run in float32r (TF32-like, full
PE rate for moving dim >= 256), accumulation in fp32 PSUM, bias+activation
fused into the ScalarE PSUM evacuation. All weights are host-packed into
[OT, 128, KT, 128] per-o-tile blocks so every weight DMA is contiguous.
"""

import math
from contextlib import ExitStack

import numpy as np

import concourse.bacc as bacc
import concourse.mybir as mybir
import concourse.tile as tile
from concourse.bass_utils import run_bass_kernel_spmd

F32 = mybir.dt.float32
F32R = mybir.dt.float32r
AF = mybir.ActivationFunctionType

D_IN, H0, H1, LAT, C = 2048, 1024, 512, 128, 8
N_CORES = 8
P = 128

# (name, K, O) for the seven dense layers, in execution order
LAYERS = [
    ("enc0", D_IN, H0),
    ("enc1", H0, H1),
    ("lv", H1, LAT),
    ("mu", H1, LAT),
    ("dec0", LAT, H1),
    ("dec1", H1, H0),
    ("out", H0, D_IN),
]
BIAS_COLS = sum(o // P for _, _, o in LAYERS)  # 42

_program_cache: dict = {}


def _nchunks(cap):
    """Split cap columns into balanced chunks, each in [256, 512] and a
    multiple of 4 (fp32r matmul ISA restriction on moving dim/offset)."""
    assert cap % 4 == 0
    k = max(1, math.ceil(cap / 512))
    if k > 1 and cap / k < 256:
        k -= 1
    q = cap // 4
    base = q // k
    rem = q - base * k
    sizes = [4 * (base + (1 if i < rem else 0)) for i in range(k)]
    assert all(256 <= s <= 512 for s in sizes) or cap < 256, (cap, sizes)
    out, acc = [], 0
    for s in sizes:
        out.append((acc, s))
        acc += s
    return out


def _build(cap):
    nbs = _nchunks(cap)
    nc = bacc.Bacc(trn_type="TRN2", target_bir_lowering=False, debug=False)

    KT_X = D_IN // P
    xh = nc.dram_tensor("xh", [P, KT_X, cap], F32R, kind="ExternalInput")
    epsT = nc.dram_tensor("epsT", [LAT, cap], F32, kind="ExternalInput")
    w_d = {
        name: nc.dram_tensor(f"w_{name}", [o // P, P, k // P, P], F32R,
                             kind="ExternalInput")
        for name, k, o in LAYERS
    }
    bias_d = nc.dram_tensor("bias_all", [P, BIAS_COLS], F32, kind="ExternalInput")

    reconT = nc.dram_tensor("reconT", [D_IN, cap], F32, kind="ExternalOutput")
    muT = nc.dram_tensor("muT", [LAT, cap], F32, kind="ExternalOutput")
    lvT = nc.dram_tensor("lvT", [LAT, cap], F32, kind="ExternalOutput")

    bias_off = {}
    acc = 0
    for name, _, o in LAYERS:
        bias_off[name] = acc
        acc += o // P

    with tile.TileContext(nc) as tc, ExitStack() as ctx:
        data = ctx.enter_context(tc.tile_pool(name="data", bufs=1))
        wstream = ctx.enter_context(tc.tile_pool(name="wstream", bufs=3))
        stage = ctx.enter_context(tc.tile_pool(name="stage", bufs=4))
        psum = ctx.enter_context(tc.tile_pool(name="psum", bufs=8, space="PSUM"))

        # x: one fully-contiguous DMA, needed first
        xt = data.tile([P, KT_X, cap], F32R, tag="slabA")
        nc.sync.dma_start(xt[:], xh[:, :, :])

        bias_sb = data.tile([P, BIAS_COLS], F32)
        nc.sync.dma_start(bias_sb[:], bias_d[:, :])

        def dense(inp, lname, act, out_dt, out_tag=None, evac=None):
            """One dense layer; weights streamed per 128-wide o-slice from the
            packed DRAM block. inp: SBUF [P, KT, cap]. Returns [P, OT, cap]
            (unless evac is given, which handles PSUM evacuation itself)."""
            _, K, O = next(l for l in LAYERS if l[0] == lname)
            KT, OT = K // P, O // P
            out = None
            if evac is None:
                out = data.tile(
                    [P, OT, cap], out_dt, name=f"a_{lname}", tag=out_tag or f"a_{lname}"
                )
            for o in range(OT):
                wt = wstream.tile([P, 16, P], F32R, tag="wstream", name=f"{lname}_w{o}")
                nc.sync.dma_start(wt[:, :KT, :], w_d[lname][o, :, :, :])
                for n0, nb in nbs:
                    ps = psum.tile([P, 512], F32, tag="ps", name=f"{lname}_ps{o}")
                    for k in range(KT):
                        nc.tensor.matmul(
                            ps[:, :nb],
                            wt[:, k, :],
                            inp[:, k, n0 : n0 + nb],
                            start=(k == 0),
                            stop=(k == KT - 1),
                        )
                    b_ap = bias_sb[:, bias_off[lname] + o : bias_off[lname] + o + 1]
                    if evac is None:
                        nc.scalar.activation(
                            out[:, o, n0 : n0 + nb], ps[:, :nb], act, bias=b_ap
                        )
                    else:
                        evac(o, n0, nb, ps, b_ap)
            return out

        # encoder
        h0 = dense(xt, "enc0", AF.Relu, F32R)
        h1 = dense(h0, "enc1", AF.Relu, F32R)

        # latent heads: lv first so the exp/z chain overlaps the mu matmuls
        lv_sb = dense(h1, "lv", AF.Identity, F32)
        t_sb = data.tile([P, cap], F32)
        nc.scalar.activation(t_sb[:], lv_sb[:, 0, :], AF.Exp, scale=0.5)
        nc.sync.dma_start(lvT[:, :], lv_sb[:, 0, :])

        mu_sb = dense(h1, "mu", AF.Identity, F32)
        nc.sync.dma_start(muT[:, :], mu_sb[:, 0, :])

        # z = mu + eps * exp(0.5 lv)
        eps_sb = data.tile([P, cap], F32)
        nc.sync.dma_start(eps_sb[:], epsT[:, :])
        z_sb = data.tile([P, 1, cap], F32R)
        nc.vector.scalar_tensor_tensor(
            z_sb[:, 0, :],
            eps_sb[:],
            1.0,
            t_sb[:],
            mybir.AluOpType.mult,
            mybir.AluOpType.mult,
        )
        nc.vector.tensor_add(z_sb[:, 0, :], z_sb[:, 0, :], mu_sb[:, 0, :])

        # decoder
        d0 = dense(z_sb, "dec0", AF.Relu, F32R, out_tag="a_enc1")
        d1 = dense(d0, "dec1", AF.Relu, F32R, out_tag="a_enc0")

        # output layer: stage through SBUF, DMA out per (o, n)
        def out_evac(o, n0, nb, ps, b_ap):
            st = stage.tile([P, 512], F32, tag="ostage", name=f"out_st{o}")
            nc.scalar.activation(st[:, :nb], ps[:, :nb], AF.Identity, bias=b_ap)
            nc.sync.dma_start(reconT[o * P : (o + 1) * P, n0 : n0 + nb], st[:, :nb])

        dense(d1, "out", AF.Identity, F32, evac=out_evac)

    nc.compile()
    return nc


def _get_program(cap):
    if cap not in _program_cache:
        _program_cache[cap] = _build(cap)
    return _program_cache[cap]


def _pack_w(w):
    """[K, O] -> [OT, 128, KT, 128] contiguous per-o-tile blocks."""
    K, O = w.shape
    return np.ascontiguousarray(
        w.reshape(K // P, P, O // P, P).transpose(2, 1, 0, 3)
    )


def _pack_b(b):
    """[O] -> [128, OT]"""
    return np.ascontiguousarray(b.reshape(-1, P).T)


def _route(cluster_labels):
    labels = np.asarray(cluster_labels).astype(np.int64)
    idx = [np.nonzero(labels == c)[0] for c in range(C)]
    cap = max(256, max((len(i) for i in idx), default=1))
    cap = (cap + 3) // 4 * 4
    idx_pad = []
    for c in range(C):
        i = idx[c]
        if len(i) == 0:
            pad = np.zeros(cap, dtype=np.int64)
        else:
            pad = np.concatenate([i, np.full(cap - len(i), i[0], dtype=np.int64)])
        idx_pad.append(pad)
    return idx, idx_pad, cap


def _run(inputs, trace=False):
    x = np.ascontiguousarray(np.asarray(inputs["x"], dtype=np.float32))
    eps = np.ascontiguousarray(np.asarray(inputs["eps"], dtype=np.float32))
    idx, idx_pad, cap = _route(inputs["cluster_labels"])
    nc = _get_program(cap)

    f32 = lambda a: np.asarray(a, dtype=np.float32)
    shared_w = {
        "w_enc1": _pack_w(f32(inputs["W_enc1"])),
        "w_lv": _pack_w(f32(inputs["W_logvar"])),
        "w_mu": _pack_w(f32(inputs["W_mu"])),
        "w_dec0": _pack_w(f32(inputs["W_dec0"])),
        "w_out": _pack_w(f32(inputs["W_out"])),
    }
    b_shared = {
        "enc1": _pack_b(f32(inputs["b_enc1"])),
        "lv": _pack_b(f32(inputs["b_logvar"])),
        "mu": _pack_b(f32(inputs["b_mu"])),
        "dec0": _pack_b(f32(inputs["b_dec0"])),
        "out": _pack_b(f32(inputs["b_out"])),
    }
    in_maps = []
    for c in range(C):
        ip = idx_pad[c]
        xT = x[ip].T  # [D_IN, cap]
        bias_all = np.concatenate(
            [
                _pack_b(f32(inputs["b_enc0"][c])),
                b_shared["enc1"],
                b_shared["lv"],
                b_shared["mu"],
                b_shared["dec0"],
                _pack_b(f32(inputs["b_dec1"][c])),
                b_shared["out"],
            ],
            axis=1,
        )
        in_maps.append(
            {
                "xh": np.ascontiguousarray(
                    xT.reshape(D_IN // P, P, cap).transpose(1, 0, 2)
                ),
                "epsT": np.ascontiguousarray(eps[ip].T),
                "w_enc0": _pack_w(f32(inputs["W_enc0"][c])),
                "w_dec1": _pack_w(f32(inputs["W_dec1"][c])),
                "bias_all": np.ascontiguousarray(bias_all),
                **shared_w,
            }
        )

    res = run_bass_kernel_spmd(nc, in_maps, core_ids=list(range(N_CORES)), trace=trace)

    B = x.shape[0]
    recon = np.empty((B, D_IN), dtype=np.float32)
    mu = np.empty((B, LAT), dtype=np.float32)
    logvar = np.empty((B, LAT), dtype=np.float32)
    for c in range(C):
        i = idx[c]
        if len(i) == 0:
            continue
        r = res.results[c]
        recon[i] = r["reconT"][:, : len(i)].T
        mu[i] = r["muT"][:, : len(i)].T
        logvar[i] = r["lvT"][:, : len(i)].T
    return (recon, mu, logvar), res


def kernel(**inputs):
    outs, _ = _run(inputs, trace=False)
    return outs
